# revision 5
# baseline (speedup 1.0000x reference)
"""Trainium2 Bass kernel for nn_Block_16621523436203 (Mamba-style block), v2.

Sharding: pure data-parallel — batch B=8, one batch element per NeuronCore,
no collectives.  Weights are preprocessed (transposed / LN-folded / cast) on
host; each core runs the full block for its batch element.

v2 restructure vs baseline:
  - all weights prefetched at kernel start (Pool-sequencer DMA issue)
  - activation functions clustered to avoid ACT table thrash
  - selective-scan phase splits the 64 scans between DVE and GPSIMD(Pool)
  - delta/du broadcast via one fused DMA per group (no PE rep-matmul)
  - B/C broadcast via DMA (no PE matmul + PSUM round-trip)
"""

import sys

sys.path.insert(0, "/opt/trn_rl_repo")

import math
import os

import ml_dtypes
import numpy as np

import concourse.bacc as bacc
import concourse.bass as bass
import concourse.mybir as mybir
import concourse.tile as tile

F32 = mybir.dt.float32
F32R = mybir.dt.float32r
BF16 = mybir.dt.bfloat16
AF = mybir.ActivationFunctionType
ALU = mybir.AluOpType

B, L, D = 8, 1024, 512
E = 1024  # d_inner
D2 = 512  # per-branch channels
R = 32  # dt_rank
NS = 16  # d_state
KC = 4  # conv kernel size
H = 2048  # mlp hidden
NCORES = 8
TT = L // 128  # 8 token tiles
DC = D // 128  # 4 d_model chunks
D2T = D2 // 128  # 4 channel tiles
ET = E // 128  # 8 d_inner tiles
HT = H // 128  # 16 hidden tiles
NG = 64  # scan groups: each = 8 channels x 16 states
EPS = 1e-5
# of each dt-tile's 16 scan groups, this many run on GPSIMD (rest on DVE)
YT_POOL = int(os.environ.get("KYTPOOL", "13"))  # yt mults per dt on GPSIMD
PEND_SKEW = int(os.environ.get("KSKEW", "2"))

# f32r pack: in_proj weight only (fp32r matmul operands must be f32r-typed)
WFR_N = DC * E                  # w_inT           [128, 4*1024]

# f32 pack column offsets (non-matmul f32 scalars/identity)
OF_IDF = 0                      # ident_f         [128, 128]
OF_APERM = OF_IDF + 128         # A_perm          [128, 64]
OF_CIN = OF_APERM + NG          # c_in            [128, 8]
OF_DTB = OF_CIN + ET            # dt_bias         [128, 4]
OF_DCOL = OF_DTB + D2T          # D_col           [128, 4]
OF_CFC1 = OF_DCOL + D2T         # c_fc1           [128, 16]
WF32_N = OF_CFC1 + HT

# bf16 pack column offsets; [0, OB_SPLIT) is the phase-1-critical head
OB_DGX = 0                      # diag_x          [128, 16*128]
OB_DGZ = OB_DGX + D2T * KC * 128
OB_XPJ = OB_DGZ + D2T * KC * 128  # x_projT       [128, 4*64]
OB_ZPAD = OB_XPJ + D2T * (R + 2 * NS)  # zpad      [128, 3]
OB_SPLIT = OB_ZPAD + 3
OB_SEL = OB_SPLIT               # sel             [128, 16*128]
OB_OPT = OB_SEL + 16 * 128      # out_projT       [128, 8*512]
OB_FC1 = OB_OPT + ET * D        # fc1T            [128, 4*2048]
OB_FC2 = OB_FC1 + DC * H        # fc2T            [128, 16*512]
OB_IDB = OB_FC2 + HT * D        # ident_bf        [128, 128]
OB_REP = OB_IDB + 128           # rep             [128, 16*128]
OB_DGD = OB_REP + 16 * 128      # diag(ssm_D)     [128, 4*128]
WBF_N = OB_DGD + D2T * 128

_BF = ml_dtypes.bfloat16


def _f32r(ap):
    return ap.bitcast(F32R)


STOP_AFTER = int(os.environ.get("KSTOP", "3"))
KREPEAT = int(os.environ.get("KREPEAT", "1"))
KALLOC = int(os.environ.get("KALLOC", "0")) or KREPEAT


def build_kernel():
    nc = bacc.Bacc("TRN2", target_bir_lowering=False, debug=False, num_devices=1)

    din = {}

    def inp(name, shape, dtype):
        din[name] = nc.dram_tensor(name, list(shape), dtype, kind="ExternalInput")
        return din[name]

    inp("xin", (KALLOC * L, D), F32)
    # host-packed weight images (see prep_inputs)
    inp("wfr", (128, WFR_N), F32R)
    inp("wf32", (128, WF32_N), F32)
    inp("wbf", (128, WBF_N), BF16)
    inp("dt_projT", (R, D2), F32R)
    inp("fc2b", (1, D), F32R)
    inp("ones1d", (1, 128), F32R)

    out_d = nc.dram_tensor("out", [KALLOC * L, D], F32, kind="ExternalOutput")

    with tile.TileContext(nc) as tc:
        for rep_i in range(KREPEAT):
            _body(nc, tc, din, out_d, rep_i * L)
    nc.compile()
    return nc


def _drain_yt(nc, scanp, sel, cbc, item):
    q, hs, ps_y, on_pool = item
    yt = scanp.tile([128, L], BF16, name="yt", tag="yt", bufs=5)
    eng = nc.gpsimd if on_pool else nc.vector
    eng.tensor_tensor(out=yt[:, :], in0=hs[:, :], in1=cbc[:, :], op=ALU.mult)
    for lc in range(2):
        nc.tensor.matmul(
            ps_y[lc][:, :],
            sel[q][:, :],
            yt[:, lc * 512 : (lc + 1) * 512],
            start=False,
            stop=(q == 15),
        )


def _body(nc, tc, din, out_d, row0=0):
    xin = din["xin"].ap()[row0 : row0 + L, :]
    out_ap = out_d.ap()[row0 : row0 + L, :]

    with (
        tc.tile_pool(name="wAll", bufs=1) as wAll,  # weights, whole kernel
        tc.tile_pool(name="p13", bufs=1) as p13,  # crosses into phase 3
    ):
        # ---------------- weight prefetch (everything, up front) -----------
        # Two big host-packed images; x tiles are queued first so LN1 can
        # start immediately, then the f32 (phase-1) pack, then the bf16 pack.
        wfr = wAll.tile([128, WFR_N], F32R, name="wfr", tag="wfr")
        wf = wAll.tile([128, WF32_N], F32, name="wf", tag="wf")
        wb = wAll.tile([128, WBF_N], BF16, name="wb", tag="wb")
        dt_projT = wAll.tile([R, D2], F32R, name="dtpj", tag="dtpj")
        fc2b = wAll.tile([1, D], F32R, name="fc2b", tag="fc2b")
        ones1 = wAll.tile([1, 128], F32R, name="ones1", tag="ones1")

        w_inT = [wfr[:, dc * E : (dc + 1) * E] for dc in range(DC)]
        diag = {
            "x": [wb[:, OB_DGX + i * 128 : OB_DGX + (i + 1) * 128] for i in range(D2T * KC)],
            "z": [wb[:, OB_DGZ + i * 128 : OB_DGZ + (i + 1) * 128] for i in range(D2T * KC)],
        }
        x_projT = [
            wb[:, OB_XPJ + dt * (R + 2 * NS) : OB_XPJ + (dt + 1) * (R + 2 * NS)]
            for dt in range(D2T)
        ]
        ident_f = wf[:, OF_IDF : OF_IDF + 128]
        a_perm = wf[:, OF_APERM : OF_APERM + NG]
        c_in = wf[:, OF_CIN : OF_CIN + ET]
        dt_bias = wf[:, OF_DTB : OF_DTB + D2T]
        d_col = wf[:, OF_DCOL : OF_DCOL + D2T]
        c_fc1 = wf[:, OF_CFC1 : OF_CFC1 + HT]
        sel = [wb[:, OB_SEL + q * 128 : OB_SEL + (q + 1) * 128] for q in range(16)]
        out_projT = [wb[:, OB_OPT + k * D : OB_OPT + (k + 1) * D] for k in range(ET)]
        fc1T = [wb[:, OB_FC1 + dc * H : OB_FC1 + (dc + 1) * H] for dc in range(DC)]
        fc2T = [wb[:, OB_FC2 + ht * D : OB_FC2 + (ht + 1) * D] for ht in range(HT)]
        ident_bf = wb[:, OB_IDB : OB_IDB + 128]
        rep = [wb[:, OB_REP + q * 128 : OB_REP + (q + 1) * 128] for q in range(16)]
        diag_d = [wb[:, OB_DGD + i * 128 : OB_DGD + (i + 1) * 128] for i in range(D2T)]
        zpad = wb[:, OB_ZPAD : OB_ZPAD + 3]

        # ---------------- cross-phase activation tensors -------------------
        zh = [p13.tile([128, L], BF16, name=f"zh{i}", tag=f"zh{i}") for i in range(D2T)]
        y_cm = [p13.tile([128, L], BF16, name=f"ycm{i}", tag=f"ycm{i}") for i in range(D2T)]
        p12_cm = tc.tile_pool(name="p12", bufs=1)  # dies after phase 2
        p12 = p12_cm.__enter__()
        xh = [p12.tile([128, L], BF16, name=f"xh{i}", tag=f"xh{i}") for i in range(D2T)]
        # ddu[dt][:, 0:L] = softplus delta (bf16); [:, L:2L] = delta*u (bf16)
        ddu = [p12.tile([128, 2 * L], BF16, name=f"ddu{i}", tag=f"ddu{i}") for i in range(D2T)]
        bbc = p12.tile([128, L], BF16, name="bbc", tag="bbc")
        cbc = p12.tile([128, L], BF16, name="cbc", tag="cbc")
        xdbl_dt = p12.tile([R, L], F32R, name="xdbl", tag="xdbl")
        eps_t = p13.tile([128, 1], F32, name="eps_t", tag="eps_t")
        nc.vector.memset(eps_t[:, :], EPS)

        # ================= PHASE 1: LN1, in_proj, conv, x_proj, dt_proj ====
        with (
            tc.tile_pool(name="xpP", bufs=1) as xpP,
            tc.tile_pool(name="t1", bufs=1) as t1,
            tc.tile_pool(name="xhatT_p", bufs=1) as xhatT_p,
            tc.tile_pool(name="psG", bufs=4, space="PSUM") as psG,
            tc.tile_pool(name="psConv", bufs=2, space="PSUM") as psConv,
            tc.tile_pool(name="psMisc", bufs=2, space="PSUM") as psMisc,
        ):
            xhatT = [xhatT_p.tile([128, L], F32R, name=f"xhT{i}", tag=f"xhT{i}") for i in range(DC)]

            # ---- LN1 (token-major); per-tile stats batched into one wide
            # Ln and one wide Exp so the act table cannot thrash ----
            x_t = [t1.tile([128, D], F32, name=f"x_t{i}", tag=f"x_t{i}") for i in range(TT)]
            for tt in range(TT):
                nc.sync.dma_start(out=x_t[tt][:, :], in_=xin[tt * 128 : (tt + 1) * 128, :])
            nc.sync.dma_start(out=wfr[:, :], in_=din["wfr"].ap()[:, :])
            nc.sync.dma_start(out=wf[:, :], in_=din["wf32"].ap()[:, :])
            nc.sync.dma_start(
                out=wb[:, 0:OB_SPLIT], in_=din["wbf"].ap()[:, 0:OB_SPLIT]
            )
            nc.sync.dma_start(
                out=wb[:, OB_SPLIT:WBF_N], in_=din["wbf"].ap()[:, OB_SPLIT:WBF_N]
            )
            nc.sync.dma_start(out=dt_projT[:, :], in_=din["dt_projT"].ap()[:, :])
            nc.sync.dma_start(out=fc2b[:, :], in_=din["fc2b"].ap()[:, :])
            nc.sync.dma_start(out=ones1[:, :], in_=din["ones1d"].ap()[:, :])
            m8 = t1.tile([128, 2 * TT], F32, name="m8", tag="m8")
            r8 = t1.tile([128, TT], F32, name="r8", tag="r8")
            lv8 = t1.tile([128, TT], F32, name="lv8", tag="lv8")
            for tt in range(TT):
                stats = t1.tile([128, 6], F32, name="stats", tag="stats", bufs=2)
                nc.vector.bn_stats(out=stats[:, :], in_=x_t[tt][:, :])
                nc.vector.bn_aggr(out=m8[:, 2 * tt : 2 * tt + 2], in_=stats[:, :])
            nc.scalar.activation(
                out=lv8[:, :], in_=m8[:, 1 : 2 * TT : 2], func=AF.Ln, bias=eps_t[:, :], scale=1.0
            )
            nc.scalar.activation(
                out=r8[:, :], in_=lv8[:, :], func=AF.Exp, bias=0.0, scale=-0.5
            )
            for tt in range(TT):
                xhat = t1.tile([128, D], F32, name="xhat", tag="xhat", bufs=3)
                nc.vector.tensor_scalar(
                    out=xhat[:, :],
                    in0=x_t[tt][:, :],
                    scalar1=m8[:, 2 * tt : 2 * tt + 1],
                    scalar2=r8[:, tt : tt + 1],
                    op0=ALU.subtract,
                    op1=ALU.mult,
                )
                for dc in range(DC):
                    ps_tr = psMisc.tile([128, 128], F32, name="ps_tr", tag="m")
                    nc.tensor.transpose(
                        ps_tr[:, :], xhat[:, dc * 128 : (dc + 1) * 128], ident_f[:, :]
                    )
                    nc.scalar.copy(out=xhatT[dc][:, tt * 128 : (tt + 1) * 128], in_=ps_tr[:, :])

            # ---- conv input buffers (padded by 1 left / 2 right) ----
            xp = {
                "x": [xpP.tile([128, L + 3], BF16, name=f"xpx{i}", tag=f"xpx{i}") for i in range(D2T)],
                "z": [xpP.tile([128, L + 3], BF16, name=f"xpz{i}", tag=f"xpz{i}") for i in range(D2T)],
            }
            for br in ("x", "z"):
                for dt in range(D2T):
                    nc.sync.dma_start(out=xp[br][dt][:, 0:1], in_=zpad[:, 0:1])
                    nc.sync.dma_start(
                        out=xp[br][dt][:, L + 1 : L + 3], in_=zpad[:, 0:2]
                    )

            # ---- in_proj: xzT[e, l] = W' @ xhatT  (+ c_in) ----
            for et in range(ET):
                for lc in range(2):
                    ps = psG.tile([128, 512], F32, name="ps_inp", tag="ps_inp")
                    for dc in range(DC):
                        nc.tensor.matmul(
                            ps[:, :],
                            w_inT[dc][:, et * 128 : (et + 1) * 128],
                            xhatT[dc][:, lc * 512 : (lc + 1) * 512],
                            start=(dc == 0),
                            stop=(dc == DC - 1),
                        )
                    br, dt = ("x", et) if et < D2T else ("z", et - D2T)
                    nc.scalar.activation(
                        out=xp[br][dt][:, 1 + lc * 512 : 1 + (lc + 1) * 512],
                        in_=ps[:, :],
                        func=AF.Identity,
                        bias=c_in[:, et : et + 1],
                        scale=1.0,
                    )

            # ---- depthwise conv (4 diagonal matmuls) + SiLU ----
            for br in ("x", "z"):
                for dt in range(D2T):
                    for lc in range(2):
                        ps = psConv.tile([128, 512], F32, name="ps_conv", tag="ps_conv")
                        for j in range(KC):
                            nc.tensor.matmul(
                                ps[:, :],
                                diag[br][dt * KC + j][:, :],
                                xp[br][dt][:, lc * 512 + j : lc * 512 + j + 512],
                                start=(j == 0),
                                stop=(j == KC - 1),
                            )
                        dst = xh[dt] if br == "x" else zh[dt]
                        nc.scalar.activation(
                            out=dst[:, lc * 512 : (lc + 1) * 512],
                            in_=ps[:, :],
                            func=AF.Silu,
                            bias=0.0,
                            scale=1.0,
                        )

            # ---- x_proj: x_dbl[r, l] = x_projT.T @ xh ----
            bc_sb = t1.tile([2 * NS, L], BF16, name="bc_sb", tag="bc_sb")
            for lc in range(2):
                ps = psMisc.tile([R + 2 * NS, 512], F32, name="ps_xdbl", tag="m")
                for dt in range(D2T):
                    nc.tensor.matmul(
                        ps[:, :],
                        x_projT[dt][:, :],
                        xh[dt][:, lc * 512 : (lc + 1) * 512],
                        start=(dt == 0),
                        stop=(dt == D2T - 1),
                    )
                nc.scalar.copy(out=xdbl_dt[:, lc * 512 : (lc + 1) * 512], in_=ps[0:R, :])
                nc.vector.tensor_copy(bc_sb[:, lc * 512 : (lc + 1) * 512], ps[R : R + 2 * NS, :])
            # broadcast B and C across partitions via DMA (p -> p % 16 pattern)
            for rep8 in range(8):
                nc.sync.dma_start(
                    out=bbc[rep8 * NS : (rep8 + 1) * NS, :], in_=bc_sb[0:NS, :]
                )
                nc.sync.dma_start(
                    out=cbc[rep8 * NS : (rep8 + 1) * NS, :], in_=bc_sb[NS : 2 * NS, :]
                )

            # ---- dt_proj + softplus -> delta(bf16) ; du = delta * xh ----
            # Two waves of (Exp x4, Ln x4) to bound live t_sp tiles; act
            # funcs stay clustered within each wave.
            for wave in range(2):
                t_sps = {}
                for dt in (2 * wave, 2 * wave + 1):
                    for lc in range(2):
                        ps = psMisc.tile([128, 512], F32, name="ps_dt", tag="m")
                        nc.tensor.matmul(
                            ps[:, :],
                            _f32r(dt_projT[:, dt * 128 : (dt + 1) * 128]),
                            _f32r(xdbl_dt[:, lc * 512 : (lc + 1) * 512]),
                            start=True,
                            stop=True,
                        )
                        t_sp = t1.tile(
                            [128, 512], F32, name=f"tsp{dt % 2}{lc}", tag=f"tsp{dt % 2}{lc}", bufs=1
                        )
                        nc.scalar.activation(
                            out=t_sp[:, :],
                            in_=ps[:, :],
                            func=AF.Exp,
                            bias=dt_bias[:, dt : dt + 1],
                            scale=1.0,
                        )
                        t_sps[(dt, lc)] = t_sp
                for dt in (2 * wave, 2 * wave + 1):
                    for lc in range(2):
                        nc.scalar.activation(
                            out=ddu[dt][:, lc * 512 : (lc + 1) * 512],
                            in_=t_sps[(dt, lc)][:, :],
                            func=AF.Ln,
                            bias=1.0,
                            scale=1.0,
                        )
                    nc.vector.tensor_tensor(
                        out=ddu[dt][:, L : 2 * L],
                        in0=ddu[dt][:, 0:L],
                        in1=xh[dt][:, :],
                        op=ALU.mult,
                    )

        if STOP_AFTER == 1:
            p12_cm.__exit__(None, None, None)
            return

        # ================= PHASE 2: selective scan ==========
        with (
            tc.tile_pool(name="scanp", bufs=7) as scanp,
            tc.tile_pool(name="psY", bufs=2, space="PSUM") as psY,
            tc.tile_pool(name="psD", bufs=3, space="PSUM") as psD,
        ):
            pend = []
            for dt in range(D2T):
                ps_y = [psY.tile([128, 512], F32, name="ps_y", tag="ps_y") for _ in range(2)]
                # seed ps_y with the D*u skip term via diag(D) matmul
                for lc in range(2):
                    nc.tensor.matmul(
                        ps_y[lc][:, :],
                        diag_d[dt][:, :],
                        xh[dt][:, lc * 512 : (lc + 1) * 512],
                        start=True,
                        stop=False,
                    )
                for q in range(16):
                    g = dt * 16 + q
                    # broadcast du rows for this group (DMA); delta rows are
                    # replicated on the PE via the rep selection matmul
                    dubc = scanp.tile([128, L], BF16, name="dubc", tag="dubc", bufs=6)
                    nc.sync.dma_start(
                        out=dubc[:, :],
                        in_=ddu[dt][q * 8 : (q + 1) * 8, L : 2 * L]
                        .unsqueeze(1)
                        .broadcast_to([8, NS, L]),
                    )
                    ps_d = psD.tile([128, L], F32, name="ps_d", tag="ps_d")
                    for lc in range(2):
                        nc.tensor.matmul(
                            ps_d[:, lc * 512 : (lc + 1) * 512],
                            rep[q][:, :],
                            ddu[dt][:, lc * 512 : (lc + 1) * 512],
                            start=True,
                            stop=True,
                        )
                    dA = scanp.tile([128, L], F32, name="dA", tag="dA", bufs=5)
                    nc.scalar.activation(
                        out=dA[:, :],
                        in_=ps_d[:, :],
                        func=AF.Exp,
                        bias=0.0,
                        scale=a_perm[:, g : g + 1],
                    )
                    dBu = scanp.tile([128, L], BF16, name="dBu", tag="dBu", bufs=6)
                    nc.vector.tensor_tensor(
                        out=dBu[:, :], in0=dubc[:, :], in1=bbc[:, :], op=ALU.mult
                    )
                    hs = scanp.tile([128, L], BF16, name="hs", tag="hs", bufs=6)
                    nc.vector.tensor_tensor_scan(
                        hs[:, :], dA[:, :], dBu[:, :], 0.0, ALU.mult, ALU.add
                    )
                    pend.append((q, hs, ps_y, q >= 16 - YT_POOL))
                    if len(pend) > PEND_SKEW:
                        _drain_yt(nc, scanp, sel, cbc, pend.pop(0))
                while pend:
                    _drain_yt(nc, scanp, sel, cbc, pend.pop(0))
                # evac: ps_y already holds y_ssm + D*u
                for lc in range(2):
                    nc.scalar.copy(
                        out=y_cm[dt][:, lc * 512 : (lc + 1) * 512], in_=ps_y[lc][:, :]
                    )

        p12_cm.__exit__(None, None, None)

        if STOP_AFTER == 2:
            for dt in range(D2T):
                nc.gpsimd.dma_start(
                    out=out_ap[dt * 128 : (dt + 1) * 128, 0:256],
                    in_=y_cm[dt][:, 0:256],
                )
            return

        # ================= PHASE 3: out_proj, LN2, MLP ==========
        with (
            tc.tile_pool(name="p3", bufs=1) as p3,
            tc.tile_pool(name="t3", bufs=3) as t3,
            tc.tile_pool(name="psG3", bufs=4, space="PSUM") as psG3,
            tc.tile_pool(name="psTr", bufs=2, space="PSUM") as psTr,
        ):
            h_res = [p3.tile([128, D], F32, name=f"hres{i}", tag=f"hres{i}") for i in range(TT)]
            xhat2 = [p3.tile([128, D], BF16, name=f"xh2{i}", tag=f"xh2{i}") for i in range(TT)]
            xhat2T = [p3.tile([128, L], BF16, name=f"xh2T{i}", tag=f"xh2T{i}") for i in range(DC)]
            aT = [p3.tile([128, L], BF16, name=f"aT{i}", tag=f"aT{i}") for i in range(HT)]
            m83 = p3.tile([128, 2 * TT], F32, name="m83", tag="m83")
            lv83 = p3.tile([128, TT], F32, name="lv83", tag="lv83")
            r83 = p3.tile([128, TT], F32, name="r83", tag="r83")

            # ---- out_proj + residual 1 + LN2 stats ----
            for tt in range(TT):
                ps = psG3.tile([128, D], F32, name="ps_op", tag="g3")
                korder = list(range(D2T, ET)) + list(range(D2T))
                for ki, k in enumerate(korder):
                    lhs = (
                        y_cm[k][:, tt * 128 : (tt + 1) * 128]
                        if k < D2T
                        else zh[k - D2T][:, tt * 128 : (tt + 1) * 128]
                    )
                    nc.tensor.matmul(
                        ps[:, :],
                        lhs,
                        out_projT[k][:, :],
                        start=(ki == 0),
                        stop=(ki == ET - 1),
                    )
                x_t = t3.tile([128, D], F32, name="x_t3", tag="x_t3")
                nc.sync.dma_start(out=x_t[:, :], in_=xin[tt * 128 : (tt + 1) * 128, :])
                nc.vector.tensor_tensor(
                    out=h_res[tt][:, :], in0=ps[:, :], in1=x_t[:, :], op=ALU.add
                )
                stats = t3.tile([128, 6], F32, name="stats3", tag="stats3")
                nc.vector.bn_stats(out=stats[:, :], in_=h_res[tt][:, :])
                nc.vector.bn_aggr(out=m83[:, 2 * tt : 2 * tt + 2], in_=stats[:, :])
            # LN2: one wide Ln + one wide Exp over all 8 tiles' stats
            nc.scalar.activation(
                out=lv83[:, :], in_=m83[:, 1 : 2 * TT : 2], func=AF.Ln, bias=eps_t[:, :], scale=1.0
            )
            nc.scalar.activation(
                out=r83[:, :], in_=lv83[:, :], func=AF.Exp, bias=0.0, scale=-0.5
            )
            for tt in range(TT):
                nc.vector.tensor_scalar(
                    out=xhat2[tt][:, :],
                    in0=h_res[tt][:, :],
                    scalar1=m83[:, 2 * tt : 2 * tt + 1],
                    scalar2=r83[:, tt : tt + 1],
                    op0=ALU.subtract,
                    op1=ALU.mult,
                )

            # ---- transpose xhat2 -> xhat2T (bf16) ----
            for dc in range(DC):
                for half in range(2):
                    ps_t = psTr.tile([128, 512], BF16, name="ps_t3", tag="ps_t3")
                    for b4 in range(4):
                        tt = half * 4 + b4
                        nc.tensor.transpose(
                            ps_t[:, b4 * 128 : (b4 + 1) * 128],
                            xhat2[tt][:, dc * 128 : (dc + 1) * 128],
                            ident_bf[:, :],
                        )
                    nc.vector.tensor_copy(
                        xhat2T[dc][:, half * 512 : (half + 1) * 512], ps_t[:, :]
                    )

            # ---- fc1 + gelu (channel-major out) ----
            for ht in range(HT):
                for lc in range(2):
                    ps = psG3.tile([128, 512], F32, name="ps_fc1", tag="g3")
                    for dc in range(DC):
                        nc.tensor.matmul(
                            ps[:, :],
                            fc1T[dc][:, ht * 128 : (ht + 1) * 128],
                            xhat2T[dc][:, lc * 512 : (lc + 1) * 512],
                            start=(dc == 0),
                            stop=(dc == DC - 1),
                        )
                    nc.scalar.activation(
                        out=aT[ht][:, lc * 512 : (lc + 1) * 512],
                        in_=ps[:, :],
                        func=AF.Gelu,
                        bias=c_fc1[:, ht : ht + 1],
                        scale=1.0,
                    )

            # ---- fc2 + bias + residual 2 -> out ----
            for tt in range(TT):
                ps = psG3.tile([128, D], F32, name="ps_fc2", tag="g3")
                for ht in range(HT):
                    nc.tensor.matmul(
                        ps[:, :],
                        aT[ht][:, tt * 128 : (tt + 1) * 128],
                        fc2T[ht][:, :],
                        start=(ht == 0),
                        stop=False,
                    )
                nc.tensor.matmul(
                    ps[:, :], ones1[:, :], fc2b[:, :], start=False, stop=True
                )
                o_t = t3.tile([128, D], F32, name="o_t", tag="o_t")
                nc.vector.tensor_tensor(
                    out=o_t[:, :], in0=ps[:, :], in1=h_res[tt][:, :], op=ALU.add
                )
                nc.sync.dma_start(out=out_ap[tt * 128 : (tt + 1) * 128, :], in_=o_t[:, :])


def prep_inputs(inputs):
    """Host-side weight preprocessing. Returns the shared (non-x) in_map."""
    g = {k: np.asarray(v, dtype=np.float32) for k, v in inputs.items()}

    ln1_w, ln1_b = g["ln1_w"], g["ln1_b"]
    ln2_w, ln2_b = g["ln2_w"], g["ln2_b"]

    w_in = g["in_proj_w"] * ln1_w[None, :]  # [E, D]
    c_in = (g["in_proj_w"] @ ln1_b).astype(np.float32)  # [E]

    fc1 = g["fc1_w"] * ln2_w[None, :]  # [H, D]
    c_fc1 = (g["fc1_w"] @ ln2_b + g["fc1_b"]).astype(np.float32)  # [H]

    A = -np.exp(g["A_log"])  # [D2, NS]
    # A_perm[p, g] = A[g*8 + p//16, p%16]
    p = np.arange(128)
    gg = np.arange(NG)
    A_perm = A[(gg[None, :] * 8 + (p // 16)[:, None]), (p % 16)[:, None]].astype(np.float32)

    # SEL[q][k, m] = 1 iff m == q*8 + k//16   (sum over n into channel rows)
    rep = np.zeros((16, 128, 128), np.float32)
    for q in range(16):
        m = np.arange(128)
        rep[q, q * 8 + m // 16, m] = 1.0
    sel = np.transpose(rep, (0, 2, 1)).copy()

    conv_x = g["conv_x_w"][:, 0, :]  # [D2, KC]
    conv_z = g["conv_z_w"][:, 0, :]
    diag_x = np.zeros((D2T * KC, 128, 128), np.float32)
    diag_z = np.zeros((D2T * KC, 128, 128), np.float32)
    idx = np.arange(128)
    for dt in range(D2T):
        for j in range(KC):
            diag_x[dt * KC + j, idx, idx] = conv_x[dt * 128 : (dt + 1) * 128, j]
            diag_z[dt * KC + j, idx, idx] = conv_z[dt * 128 : (dt + 1) * 128, j]

    def bf(x):
        return np.ascontiguousarray(x.astype(_BF))

    # [D, E] -> [128, DC*E] with block dc holding rows dc*128..dc*128+127
    def packrows(a, blk):
        n, m = a.shape
        k = n // 128
        outw = np.zeros((128, k * m), a.dtype)
        for i in range(k):
            outw[:, i * m : (i + 1) * m] = a[i * 128 : (i + 1) * 128, :]
        return outw

    wfr = packrows(w_in.T.astype(np.float32), DC)

    wf32 = np.zeros((128, WF32_N), np.float32)
    wf32[:, OF_IDF : OF_IDF + 128] = np.eye(128, dtype=np.float32)
    wf32[:, OF_APERM : OF_APERM + NG] = A_perm
    wf32[:, OF_CIN : OF_CIN + ET] = c_in.reshape(ET, 128).T
    wf32[:, OF_DTB : OF_DTB + D2T] = g["dt_proj_b"].reshape(D2T, 128).T
    wf32[:, OF_DCOL : OF_DCOL + D2T] = g["ssm_D"].reshape(D2T, 128).T
    wf32[:, OF_CFC1 : OF_CFC1 + HT] = c_fc1.reshape(HT, 128).T

    wbf = np.zeros((128, WBF_N), _BF)
    for i in range(D2T * KC):
        wbf[:, OB_DGX + i * 128 : OB_DGX + (i + 1) * 128] = diag_x[i].astype(_BF)
        wbf[:, OB_DGZ + i * 128 : OB_DGZ + (i + 1) * 128] = diag_z[i].astype(_BF)
    wbf[:, OB_XPJ : OB_XPJ + D2T * (R + 2 * NS)] = packrows(
        bf(g["x_proj_w"].T), D2T
    )
    for q in range(16):
        wbf[:, OB_SEL + q * 128 : OB_SEL + (q + 1) * 128] = sel[q].astype(_BF)
    wbf[:, OB_OPT : OB_OPT + ET * D] = packrows(bf(g["out_proj_w"].T), ET)
    wbf[:, OB_FC1 : OB_FC1 + DC * H] = packrows(bf(fc1.T), DC)
    wbf[:, OB_FC2 : OB_FC2 + HT * D] = packrows(bf(g["fc2_w"].T), HT)
    wbf[:, OB_IDB : OB_IDB + 128] = np.eye(128, dtype=np.float32).astype(_BF)
    for q in range(16):
        wbf[:, OB_REP + q * 128 : OB_REP + (q + 1) * 128] = rep[q].astype(_BF)
    dcol = g["ssm_D"].reshape(D2T, 128)
    for i in range(D2T):
        wbf[:, OB_DGD + i * 128 : OB_DGD + (i + 1) * 128] = np.diag(dcol[i]).astype(_BF)
    # zpad cols stay zero

    f = np.ascontiguousarray
    shared = {
        "wfr": f(wfr),
        "wf32": f(wf32),
        "wbf": f(wbf),
        "dt_projT": f(g["dt_proj_w"].T),
        "fc2b": f(g["fc2_b"].reshape(1, D)),
        "ones1d": np.ones((1, 128), np.float32),
    }
    return shared


_CACHED_NC = None


def kernel(**inputs):
    global _CACHED_NC
    from concourse.bass_utils import run_bass_kernel_spmd

    if _CACHED_NC is None:
        _CACHED_NC = build_kernel()
    nc = _CACHED_NC

    shared = prep_inputs(inputs)
    x = np.asarray(inputs["x"], dtype=np.float32)
    in_maps = [
        dict(shared, xin=np.ascontiguousarray(np.concatenate([x[i]] * KREPEAT, axis=0)))
        for i in range(NCORES)
    ]
    res = run_bass_kernel_spmd(nc, in_maps, core_ids=list(range(NCORES)))
    out = np.stack([res.results[i]["out"][:L] for i in range(NCORES)], axis=0)
    return out


if __name__ == "__main__":
    nc = build_kernel()
    print("build ok")


# revision 6
# speedup vs baseline: 1.0085x; 1.0085x over previous
"""Trainium2 Bass kernel for nn_Block_16621523436203 (Mamba-style block), v2.

Sharding: pure data-parallel — batch B=8, one batch element per NeuronCore,
no collectives.  Weights are preprocessed (transposed / LN-folded / cast) on
host; each core runs the full block for its batch element.

v2 restructure vs baseline:
  - all weights prefetched at kernel start (Pool-sequencer DMA issue)
  - activation functions clustered to avoid ACT table thrash
  - selective-scan phase splits the 64 scans between DVE and GPSIMD(Pool)
  - delta/du broadcast via one fused DMA per group (no PE rep-matmul)
  - B/C broadcast via DMA (no PE matmul + PSUM round-trip)
"""

import sys

sys.path.insert(0, "/opt/trn_rl_repo")

import math
import os

import ml_dtypes
import numpy as np

import concourse.bacc as bacc
import concourse.bass as bass
import concourse.mybir as mybir
import concourse.tile as tile

F32 = mybir.dt.float32
F32R = mybir.dt.float32r
BF16 = mybir.dt.bfloat16
AF = mybir.ActivationFunctionType
ALU = mybir.AluOpType

B, L, D = 8, 1024, 512
E = 1024  # d_inner
D2 = 512  # per-branch channels
R = 32  # dt_rank
NS = 16  # d_state
KC = 4  # conv kernel size
H = 2048  # mlp hidden
NCORES = 8
TT = L // 128  # 8 token tiles
DC = D // 128  # 4 d_model chunks
D2T = D2 // 128  # 4 channel tiles
ET = E // 128  # 8 d_inner tiles
HT = H // 128  # 16 hidden tiles
NG = 64  # scan groups: each = 8 channels x 16 states
EPS = 1e-5
# of each dt-tile's 16 scan groups, this many run on GPSIMD (rest on DVE)
YT_POOL = int(os.environ.get("KYTPOOL", "13"))  # yt mults per dt on GPSIMD
PEND_SKEW = int(os.environ.get("KSKEW", "4"))

# f32r pack: in_proj weight only (fp32r matmul operands must be f32r-typed)
WFR_N = DC * E                  # w_inT           [128, 4*1024]

# f32 pack column offsets (non-matmul f32 scalars/identity)
OF_IDF = 0                      # ident_f         [128, 128]
OF_APERM = OF_IDF + 128         # A_perm          [128, 64]
OF_CIN = OF_APERM + NG          # c_in            [128, 8]
OF_DTB = OF_CIN + ET            # dt_bias         [128, 4]
OF_DCOL = OF_DTB + D2T          # D_col           [128, 4]
OF_CFC1 = OF_DCOL + D2T         # c_fc1           [128, 16]
WF32_N = OF_CFC1 + HT

# bf16 pack column offsets; [0, OB_SPLIT) is the phase-1-critical head
OB_DGX = 0                      # diag_x          [128, 16*128]
OB_DGZ = OB_DGX + D2T * KC * 128
OB_XPJ = OB_DGZ + D2T * KC * 128  # x_projT       [128, 4*64]
OB_ZPAD = OB_XPJ + D2T * (R + 2 * NS)  # zpad      [128, 3]
OB_SPLIT = OB_ZPAD + 3
OB_SEL = OB_SPLIT               # sel             [128, 16*128]
OB_OPT = OB_SEL + 16 * 128      # out_projT       [128, 8*512]
OB_FC1 = OB_OPT + ET * D        # fc1T            [128, 4*2048]
OB_FC2 = OB_FC1 + DC * H        # fc2T            [128, 16*512]
OB_IDB = OB_FC2 + HT * D        # ident_bf        [128, 128]
OB_REP = OB_IDB + 128           # rep             [128, 16*128]
OB_DGD = OB_REP + 16 * 128      # diag(ssm_D)     [128, 4*128]
WBF_N = OB_DGD + D2T * 128

_BF = ml_dtypes.bfloat16


def _f32r(ap):
    return ap.bitcast(F32R)


STOP_AFTER = int(os.environ.get("KSTOP", "3"))
KREPEAT = int(os.environ.get("KREPEAT", "1"))
KALLOC = int(os.environ.get("KALLOC", "0")) or KREPEAT


def build_kernel():
    nc = bacc.Bacc("TRN2", target_bir_lowering=False, debug=False, num_devices=1)

    din = {}

    def inp(name, shape, dtype):
        din[name] = nc.dram_tensor(name, list(shape), dtype, kind="ExternalInput")
        return din[name]

    inp("xin", (KALLOC * L, D), F32)
    # host-packed weight images (see prep_inputs)
    inp("wfr", (128, WFR_N), F32R)
    inp("wf32", (128, WF32_N), F32)
    inp("wbf", (128, WBF_N), BF16)
    inp("dt_projT", (R, D2), F32R)
    inp("fc2b", (1, D), F32R)
    inp("ones1d", (1, 128), F32R)

    out_d = nc.dram_tensor("out", [KALLOC * L, D], F32, kind="ExternalOutput")

    with tile.TileContext(nc) as tc:
        for rep_i in range(KREPEAT):
            _body(nc, tc, din, out_d, rep_i * L)
    nc.compile()
    return nc


def _drain_yt(nc, scanp, sel, cbc, item):
    q, hs, ps_y, on_pool = item
    yt = scanp.tile([128, L], BF16, name="yt", tag="yt", bufs=5)
    eng = nc.gpsimd if on_pool else nc.vector
    eng.tensor_tensor(out=yt[:, :], in0=hs[:, :], in1=cbc[:, :], op=ALU.mult)
    for lc in range(2):
        nc.tensor.matmul(
            ps_y[lc][:, :],
            sel[q][:, :],
            yt[:, lc * 512 : (lc + 1) * 512],
            start=False,
            stop=(q == 15),
        )


def _body(nc, tc, din, out_d, row0=0):
    xin = din["xin"].ap()[row0 : row0 + L, :]
    out_ap = out_d.ap()[row0 : row0 + L, :]

    with (
        tc.tile_pool(name="wAll", bufs=1) as wAll,  # weights, whole kernel
        tc.tile_pool(name="p13", bufs=1) as p13,  # crosses into phase 3
    ):
        # ---------------- weight prefetch (everything, up front) -----------
        # Two big host-packed images; x tiles are queued first so LN1 can
        # start immediately, then the f32 (phase-1) pack, then the bf16 pack.
        wfr = wAll.tile([128, WFR_N], F32R, name="wfr", tag="wfr")
        wf = wAll.tile([128, WF32_N], F32, name="wf", tag="wf")
        wb = wAll.tile([128, WBF_N], BF16, name="wb", tag="wb")
        dt_projT = wAll.tile([R, D2], F32R, name="dtpj", tag="dtpj")
        fc2b = wAll.tile([1, D], F32R, name="fc2b", tag="fc2b")
        ones1 = wAll.tile([1, 128], F32R, name="ones1", tag="ones1")

        w_inT = [wfr[:, dc * E : (dc + 1) * E] for dc in range(DC)]
        diag = {
            "x": [wb[:, OB_DGX + i * 128 : OB_DGX + (i + 1) * 128] for i in range(D2T * KC)],
            "z": [wb[:, OB_DGZ + i * 128 : OB_DGZ + (i + 1) * 128] for i in range(D2T * KC)],
        }
        x_projT = [
            wb[:, OB_XPJ + dt * (R + 2 * NS) : OB_XPJ + (dt + 1) * (R + 2 * NS)]
            for dt in range(D2T)
        ]
        ident_f = wf[:, OF_IDF : OF_IDF + 128]
        a_perm = wf[:, OF_APERM : OF_APERM + NG]
        c_in = wf[:, OF_CIN : OF_CIN + ET]
        dt_bias = wf[:, OF_DTB : OF_DTB + D2T]
        d_col = wf[:, OF_DCOL : OF_DCOL + D2T]
        c_fc1 = wf[:, OF_CFC1 : OF_CFC1 + HT]
        sel = [wb[:, OB_SEL + q * 128 : OB_SEL + (q + 1) * 128] for q in range(16)]
        out_projT = [wb[:, OB_OPT + k * D : OB_OPT + (k + 1) * D] for k in range(ET)]
        fc1T = [wb[:, OB_FC1 + dc * H : OB_FC1 + (dc + 1) * H] for dc in range(DC)]
        fc2T = [wb[:, OB_FC2 + ht * D : OB_FC2 + (ht + 1) * D] for ht in range(HT)]
        ident_bf = wb[:, OB_IDB : OB_IDB + 128]
        rep = [wb[:, OB_REP + q * 128 : OB_REP + (q + 1) * 128] for q in range(16)]
        diag_d = [wb[:, OB_DGD + i * 128 : OB_DGD + (i + 1) * 128] for i in range(D2T)]
        zpad = wb[:, OB_ZPAD : OB_ZPAD + 3]

        # ---------------- cross-phase activation tensors -------------------
        zh = [p13.tile([128, L], BF16, name=f"zh{i}", tag=f"zh{i}") for i in range(D2T)]
        y_cm = [p13.tile([128, L], BF16, name=f"ycm{i}", tag=f"ycm{i}") for i in range(D2T)]
        p12_cm = tc.tile_pool(name="p12", bufs=1)  # dies after phase 2
        p12 = p12_cm.__enter__()
        xh = [p12.tile([128, L], BF16, name=f"xh{i}", tag=f"xh{i}") for i in range(D2T)]
        # ddu[dt][:, 0:L] = softplus delta (bf16); [:, L:2L] = delta*u (bf16)
        ddu = [p12.tile([128, 2 * L], BF16, name=f"ddu{i}", tag=f"ddu{i}") for i in range(D2T)]
        bbc = p12.tile([128, L], BF16, name="bbc", tag="bbc")
        cbc = p12.tile([128, L], BF16, name="cbc", tag="cbc")
        xdbl_dt = p12.tile([R, L], F32R, name="xdbl", tag="xdbl")
        eps_t = p13.tile([128, 1], F32, name="eps_t", tag="eps_t")
        nc.vector.memset(eps_t[:, :], EPS)

        # ================= PHASE 1: LN1, in_proj, conv, x_proj, dt_proj ====
        with (
            tc.tile_pool(name="xpP", bufs=1) as xpP,
            tc.tile_pool(name="t1", bufs=1) as t1,
            tc.tile_pool(name="xhatT_p", bufs=1) as xhatT_p,
            tc.tile_pool(name="psG", bufs=4, space="PSUM") as psG,
            tc.tile_pool(name="psConv", bufs=2, space="PSUM") as psConv,
            tc.tile_pool(name="psMisc", bufs=2, space="PSUM") as psMisc,
        ):
            xhatT = [xhatT_p.tile([128, L], F32R, name=f"xhT{i}", tag=f"xhT{i}") for i in range(DC)]

            # ---- LN1 (token-major); per-tile stats batched into one wide
            # Ln and one wide Exp so the act table cannot thrash ----
            x_t = [t1.tile([128, D], F32, name=f"x_t{i}", tag=f"x_t{i}") for i in range(TT)]
            for tt in range(TT):
                nc.sync.dma_start(out=x_t[tt][:, :], in_=xin[tt * 128 : (tt + 1) * 128, :])
            nc.sync.dma_start(out=wfr[:, :], in_=din["wfr"].ap()[:, :])
            nc.sync.dma_start(out=wf[:, :], in_=din["wf32"].ap()[:, :])
            nc.sync.dma_start(
                out=wb[:, 0:OB_SPLIT], in_=din["wbf"].ap()[:, 0:OB_SPLIT]
            )
            nc.sync.dma_start(
                out=wb[:, OB_SPLIT:WBF_N], in_=din["wbf"].ap()[:, OB_SPLIT:WBF_N]
            )
            nc.sync.dma_start(out=dt_projT[:, :], in_=din["dt_projT"].ap()[:, :])
            nc.sync.dma_start(out=fc2b[:, :], in_=din["fc2b"].ap()[:, :])
            nc.sync.dma_start(out=ones1[:, :], in_=din["ones1d"].ap()[:, :])
            m8 = t1.tile([128, 2 * TT], F32, name="m8", tag="m8")
            r8 = t1.tile([128, TT], F32, name="r8", tag="r8")
            lv8 = t1.tile([128, TT], F32, name="lv8", tag="lv8")
            for tt in range(TT):
                stats = t1.tile([128, 6], F32, name="stats", tag="stats", bufs=2)
                nc.vector.bn_stats(out=stats[:, :], in_=x_t[tt][:, :])
                nc.vector.bn_aggr(out=m8[:, 2 * tt : 2 * tt + 2], in_=stats[:, :])
            nc.scalar.activation(
                out=lv8[:, :], in_=m8[:, 1 : 2 * TT : 2], func=AF.Ln, bias=eps_t[:, :], scale=1.0
            )
            nc.scalar.activation(
                out=r8[:, :], in_=lv8[:, :], func=AF.Exp, bias=0.0, scale=-0.5
            )
            for tt in range(TT):
                xhat = t1.tile([128, D], F32, name="xhat", tag="xhat", bufs=3)
                nc.vector.tensor_scalar(
                    out=xhat[:, :],
                    in0=x_t[tt][:, :],
                    scalar1=m8[:, 2 * tt : 2 * tt + 1],
                    scalar2=r8[:, tt : tt + 1],
                    op0=ALU.subtract,
                    op1=ALU.mult,
                )
                for dc in range(DC):
                    ps_tr = psMisc.tile([128, 128], F32, name="ps_tr", tag="m")
                    nc.tensor.transpose(
                        ps_tr[:, :], xhat[:, dc * 128 : (dc + 1) * 128], ident_f[:, :]
                    )
                    nc.scalar.copy(out=xhatT[dc][:, tt * 128 : (tt + 1) * 128], in_=ps_tr[:, :])

            # ---- conv input buffers (padded by 1 left / 2 right) ----
            xp = {
                "x": [xpP.tile([128, L + 3], BF16, name=f"xpx{i}", tag=f"xpx{i}") for i in range(D2T)],
                "z": [xpP.tile([128, L + 3], BF16, name=f"xpz{i}", tag=f"xpz{i}") for i in range(D2T)],
            }
            for br in ("x", "z"):
                for dt in range(D2T):
                    nc.sync.dma_start(out=xp[br][dt][:, 0:1], in_=zpad[:, 0:1])
                    nc.sync.dma_start(
                        out=xp[br][dt][:, L + 1 : L + 3], in_=zpad[:, 0:2]
                    )

            # ---- in_proj: xzT[e, l] = W' @ xhatT  (+ c_in) ----
            for et in range(ET):
                for lc in range(2):
                    ps = psG.tile([128, 512], F32, name="ps_inp", tag="ps_inp")
                    for dc in range(DC):
                        nc.tensor.matmul(
                            ps[:, :],
                            w_inT[dc][:, et * 128 : (et + 1) * 128],
                            xhatT[dc][:, lc * 512 : (lc + 1) * 512],
                            start=(dc == 0),
                            stop=(dc == DC - 1),
                        )
                    br, dt = ("x", et) if et < D2T else ("z", et - D2T)
                    nc.vector.tensor_scalar(
                        out=xp[br][dt][:, 1 + lc * 512 : 1 + (lc + 1) * 512],
                        in0=ps[:, :],
                        scalar1=c_in[:, et : et + 1],
                        scalar2=None,
                        op0=ALU.add,
                    )

            # ---- depthwise conv (4 diagonal matmuls) + SiLU ----
            for br in ("x", "z"):
                for dt in range(D2T):
                    for lc in range(2):
                        ps = psConv.tile([128, 512], F32, name="ps_conv", tag="ps_conv")
                        for j in range(KC):
                            nc.tensor.matmul(
                                ps[:, :],
                                diag[br][dt * KC + j][:, :],
                                xp[br][dt][:, lc * 512 + j : lc * 512 + j + 512],
                                start=(j == 0),
                                stop=(j == KC - 1),
                            )
                        dst = xh[dt] if br == "x" else zh[dt]
                        nc.scalar.activation(
                            out=dst[:, lc * 512 : (lc + 1) * 512],
                            in_=ps[:, :],
                            func=AF.Silu,
                            bias=0.0,
                            scale=1.0,
                        )

            # ---- x_proj: x_dbl[r, l] = x_projT.T @ xh ----
            bc_sb = t1.tile([2 * NS, L], BF16, name="bc_sb", tag="bc_sb")
            for lc in range(2):
                ps = psMisc.tile([R + 2 * NS, 512], F32, name="ps_xdbl", tag="m")
                for dt in range(D2T):
                    nc.tensor.matmul(
                        ps[:, :],
                        x_projT[dt][:, :],
                        xh[dt][:, lc * 512 : (lc + 1) * 512],
                        start=(dt == 0),
                        stop=(dt == D2T - 1),
                    )
                nc.scalar.copy(out=xdbl_dt[:, lc * 512 : (lc + 1) * 512], in_=ps[0:R, :])
                nc.vector.tensor_copy(bc_sb[:, lc * 512 : (lc + 1) * 512], ps[R : R + 2 * NS, :])
            # broadcast B and C across partitions via DMA (p -> p % 16 pattern)
            for rep8 in range(8):
                nc.sync.dma_start(
                    out=bbc[rep8 * NS : (rep8 + 1) * NS, :], in_=bc_sb[0:NS, :]
                )
                nc.sync.dma_start(
                    out=cbc[rep8 * NS : (rep8 + 1) * NS, :], in_=bc_sb[NS : 2 * NS, :]
                )

            # ---- dt_proj + softplus -> delta(bf16) ; du = delta * xh ----
            # Two waves of (Exp x4, Ln x4) to bound live t_sp tiles; act
            # funcs stay clustered within each wave.
            for wave in range(2):
                t_sps = {}
                for dt in (2 * wave, 2 * wave + 1):
                    for lc in range(2):
                        ps = psMisc.tile([128, 512], F32, name="ps_dt", tag="m")
                        nc.tensor.matmul(
                            ps[:, :],
                            _f32r(dt_projT[:, dt * 128 : (dt + 1) * 128]),
                            _f32r(xdbl_dt[:, lc * 512 : (lc + 1) * 512]),
                            start=True,
                            stop=True,
                        )
                        t_sp = t1.tile(
                            [128, 512], F32, name=f"tsp{dt % 2}{lc}", tag=f"tsp{dt % 2}{lc}", bufs=1
                        )
                        nc.scalar.activation(
                            out=t_sp[:, :],
                            in_=ps[:, :],
                            func=AF.Exp,
                            bias=dt_bias[:, dt : dt + 1],
                            scale=1.0,
                        )
                        t_sps[(dt, lc)] = t_sp
                for dt in (2 * wave, 2 * wave + 1):
                    for lc in range(2):
                        nc.scalar.activation(
                            out=ddu[dt][:, lc * 512 : (lc + 1) * 512],
                            in_=t_sps[(dt, lc)][:, :],
                            func=AF.Ln,
                            bias=1.0,
                            scale=1.0,
                        )
                    nc.vector.tensor_tensor(
                        out=ddu[dt][:, L : 2 * L],
                        in0=ddu[dt][:, 0:L],
                        in1=xh[dt][:, :],
                        op=ALU.mult,
                    )

        if STOP_AFTER == 1:
            p12_cm.__exit__(None, None, None)
            return

        # ================= PHASE 2: selective scan ==========
        with (
            tc.tile_pool(name="scanp", bufs=7) as scanp,
            tc.tile_pool(name="psY", bufs=int(os.environ.get("KPSY", "2")), space="PSUM") as psY,
            tc.tile_pool(name="psD", bufs=int(os.environ.get("KPSD", "3")), space="PSUM") as psD,
        ):
            pend = []
            for dt in range(D2T):
                ps_y = [psY.tile([128, 512], F32, name="ps_y", tag="ps_y") for _ in range(2)]
                # seed ps_y with the D*u skip term via diag(D) matmul
                for lc in range(2):
                    nc.tensor.matmul(
                        ps_y[lc][:, :],
                        diag_d[dt][:, :],
                        xh[dt][:, lc * 512 : (lc + 1) * 512],
                        start=True,
                        stop=False,
                    )
                for q in range(16):
                    g = dt * 16 + q
                    # broadcast du rows for this group (DMA); delta rows are
                    # replicated on the PE via the rep selection matmul
                    dubc = scanp.tile([128, L], BF16, name="dubc", tag="dubc", bufs=6)
                    nc.sync.dma_start(
                        out=dubc[:, :],
                        in_=ddu[dt][q * 8 : (q + 1) * 8, L : 2 * L]
                        .unsqueeze(1)
                        .broadcast_to([8, NS, L]),
                    )
                    ps_d = psD.tile([128, L], F32, name="ps_d", tag="ps_d")
                    for lc in range(2):
                        nc.tensor.matmul(
                            ps_d[:, lc * 512 : (lc + 1) * 512],
                            rep[q][:, :],
                            ddu[dt][:, lc * 512 : (lc + 1) * 512],
                            start=True,
                            stop=True,
                        )
                    dA = scanp.tile([128, L], F32, name="dA", tag="dA", bufs=5)
                    nc.scalar.activation(
                        out=dA[:, :],
                        in_=ps_d[:, :],
                        func=AF.Exp,
                        bias=0.0,
                        scale=a_perm[:, g : g + 1],
                    )
                    dBu = scanp.tile([128, L], BF16, name="dBu", tag="dBu", bufs=6)
                    nc.vector.tensor_tensor(
                        out=dBu[:, :], in0=dubc[:, :], in1=bbc[:, :], op=ALU.mult
                    )
                    hs = scanp.tile([128, L], BF16, name="hs", tag="hs", bufs=6)
                    nc.vector.tensor_tensor_scan(
                        hs[:, :], dA[:, :], dBu[:, :], 0.0, ALU.mult, ALU.add
                    )
                    pend.append((q, hs, ps_y, q >= 16 - YT_POOL))
                    if len(pend) > PEND_SKEW:
                        _drain_yt(nc, scanp, sel, cbc, pend.pop(0))
                while pend:
                    _drain_yt(nc, scanp, sel, cbc, pend.pop(0))
                # evac: ps_y already holds y_ssm + D*u
                for lc in range(2):
                    nc.scalar.copy(
                        out=y_cm[dt][:, lc * 512 : (lc + 1) * 512], in_=ps_y[lc][:, :]
                    )

        p12_cm.__exit__(None, None, None)

        if STOP_AFTER == 2:
            for dt in range(D2T):
                nc.gpsimd.dma_start(
                    out=out_ap[dt * 128 : (dt + 1) * 128, 0:256],
                    in_=y_cm[dt][:, 0:256],
                )
            return

        # ================= PHASE 3: out_proj, LN2, MLP ==========
        with (
            tc.tile_pool(name="p3", bufs=1) as p3,
            tc.tile_pool(name="t3", bufs=3) as t3,
            tc.tile_pool(name="psG3", bufs=4, space="PSUM") as psG3,
            tc.tile_pool(name="psTr", bufs=2, space="PSUM") as psTr,
        ):
            h_res = [p3.tile([128, D], F32, name=f"hres{i}", tag=f"hres{i}") for i in range(TT)]
            xhat2 = [p3.tile([128, D], BF16, name=f"xh2{i}", tag=f"xh2{i}") for i in range(TT)]
            xhat2T = [p3.tile([128, L], BF16, name=f"xh2T{i}", tag=f"xh2T{i}") for i in range(DC)]
            aT = [p3.tile([128, L], BF16, name=f"aT{i}", tag=f"aT{i}") for i in range(HT)]
            m83 = p3.tile([128, 2 * TT], F32, name="m83", tag="m83")
            lv83 = p3.tile([128, TT], F32, name="lv83", tag="lv83")
            r83 = p3.tile([128, TT], F32, name="r83", tag="r83")

            # ---- out_proj + residual 1 + LN2 stats ----
            for tt in range(TT):
                ps = psG3.tile([128, D], F32, name="ps_op", tag="g3")
                korder = list(range(D2T, ET)) + list(range(D2T))
                for ki, k in enumerate(korder):
                    lhs = (
                        y_cm[k][:, tt * 128 : (tt + 1) * 128]
                        if k < D2T
                        else zh[k - D2T][:, tt * 128 : (tt + 1) * 128]
                    )
                    nc.tensor.matmul(
                        ps[:, :],
                        lhs,
                        out_projT[k][:, :],
                        start=(ki == 0),
                        stop=(ki == ET - 1),
                    )
                x_t = t3.tile([128, D], F32, name="x_t3", tag="x_t3")
                nc.sync.dma_start(out=x_t[:, :], in_=xin[tt * 128 : (tt + 1) * 128, :])
                nc.vector.tensor_tensor(
                    out=h_res[tt][:, :], in0=ps[:, :], in1=x_t[:, :], op=ALU.add
                )
                stats = t3.tile([128, 6], F32, name="stats3", tag="stats3")
                nc.vector.bn_stats(out=stats[:, :], in_=h_res[tt][:, :])
                nc.vector.bn_aggr(out=m83[:, 2 * tt : 2 * tt + 2], in_=stats[:, :])
            # LN2: one wide Ln + one wide Exp over all 8 tiles' stats
            nc.scalar.activation(
                out=lv83[:, :], in_=m83[:, 1 : 2 * TT : 2], func=AF.Ln, bias=eps_t[:, :], scale=1.0
            )
            nc.scalar.activation(
                out=r83[:, :], in_=lv83[:, :], func=AF.Exp, bias=0.0, scale=-0.5
            )
            for tt in range(TT):
                nc.vector.tensor_scalar(
                    out=xhat2[tt][:, :],
                    in0=h_res[tt][:, :],
                    scalar1=m83[:, 2 * tt : 2 * tt + 1],
                    scalar2=r83[:, tt : tt + 1],
                    op0=ALU.subtract,
                    op1=ALU.mult,
                )

            # ---- transpose xhat2 -> xhat2T (bf16) ----
            for dc in range(DC):
                for half in range(2):
                    ps_t = psTr.tile([128, 512], BF16, name="ps_t3", tag="ps_t3")
                    for b4 in range(4):
                        tt = half * 4 + b4
                        nc.tensor.transpose(
                            ps_t[:, b4 * 128 : (b4 + 1) * 128],
                            xhat2[tt][:, dc * 128 : (dc + 1) * 128],
                            ident_bf[:, :],
                        )
                    nc.vector.tensor_copy(
                        xhat2T[dc][:, half * 512 : (half + 1) * 512], ps_t[:, :]
                    )

            # ---- fc1 + gelu (channel-major out) ----
            for ht in range(HT):
                for lc in range(2):
                    ps = psG3.tile([128, 512], F32, name="ps_fc1", tag="g3")
                    for dc in range(DC):
                        nc.tensor.matmul(
                            ps[:, :],
                            fc1T[dc][:, ht * 128 : (ht + 1) * 128],
                            xhat2T[dc][:, lc * 512 : (lc + 1) * 512],
                            start=(dc == 0),
                            stop=(dc == DC - 1),
                        )
                    nc.scalar.activation(
                        out=aT[ht][:, lc * 512 : (lc + 1) * 512],
                        in_=ps[:, :],
                        func=AF.Gelu,
                        bias=c_fc1[:, ht : ht + 1],
                        scale=1.0,
                    )

            # ---- fc2 + bias + residual 2 -> out ----
            for tt in range(TT):
                ps = psG3.tile([128, D], F32, name="ps_fc2", tag="g3")
                for ht in range(HT):
                    nc.tensor.matmul(
                        ps[:, :],
                        aT[ht][:, tt * 128 : (tt + 1) * 128],
                        fc2T[ht][:, :],
                        start=(ht == 0),
                        stop=False,
                    )
                nc.tensor.matmul(
                    ps[:, :], ones1[:, :], fc2b[:, :], start=False, stop=True
                )
                o_t = t3.tile([128, D], F32, name="o_t", tag="o_t")
                nc.vector.tensor_tensor(
                    out=o_t[:, :], in0=ps[:, :], in1=h_res[tt][:, :], op=ALU.add
                )
                nc.sync.dma_start(out=out_ap[tt * 128 : (tt + 1) * 128, :], in_=o_t[:, :])


def prep_inputs(inputs):
    """Host-side weight preprocessing. Returns the shared (non-x) in_map."""
    g = {k: np.asarray(v, dtype=np.float32) for k, v in inputs.items()}

    ln1_w, ln1_b = g["ln1_w"], g["ln1_b"]
    ln2_w, ln2_b = g["ln2_w"], g["ln2_b"]

    w_in = g["in_proj_w"] * ln1_w[None, :]  # [E, D]
    c_in = (g["in_proj_w"] @ ln1_b).astype(np.float32)  # [E]

    fc1 = g["fc1_w"] * ln2_w[None, :]  # [H, D]
    c_fc1 = (g["fc1_w"] @ ln2_b + g["fc1_b"]).astype(np.float32)  # [H]

    A = -np.exp(g["A_log"])  # [D2, NS]
    # A_perm[p, g] = A[g*8 + p//16, p%16]
    p = np.arange(128)
    gg = np.arange(NG)
    A_perm = A[(gg[None, :] * 8 + (p // 16)[:, None]), (p % 16)[:, None]].astype(np.float32)

    # SEL[q][k, m] = 1 iff m == q*8 + k//16   (sum over n into channel rows)
    rep = np.zeros((16, 128, 128), np.float32)
    for q in range(16):
        m = np.arange(128)
        rep[q, q * 8 + m // 16, m] = 1.0
    sel = np.transpose(rep, (0, 2, 1)).copy()

    conv_x = g["conv_x_w"][:, 0, :]  # [D2, KC]
    conv_z = g["conv_z_w"][:, 0, :]
    diag_x = np.zeros((D2T * KC, 128, 128), np.float32)
    diag_z = np.zeros((D2T * KC, 128, 128), np.float32)
    idx = np.arange(128)
    for dt in range(D2T):
        for j in range(KC):
            diag_x[dt * KC + j, idx, idx] = conv_x[dt * 128 : (dt + 1) * 128, j]
            diag_z[dt * KC + j, idx, idx] = conv_z[dt * 128 : (dt + 1) * 128, j]

    def bf(x):
        return np.ascontiguousarray(x.astype(_BF))

    # [D, E] -> [128, DC*E] with block dc holding rows dc*128..dc*128+127
    def packrows(a, blk):
        n, m = a.shape
        k = n // 128
        outw = np.zeros((128, k * m), a.dtype)
        for i in range(k):
            outw[:, i * m : (i + 1) * m] = a[i * 128 : (i + 1) * 128, :]
        return outw

    wfr = packrows(w_in.T.astype(np.float32), DC)

    wf32 = np.zeros((128, WF32_N), np.float32)
    wf32[:, OF_IDF : OF_IDF + 128] = np.eye(128, dtype=np.float32)
    wf32[:, OF_APERM : OF_APERM + NG] = A_perm
    wf32[:, OF_CIN : OF_CIN + ET] = c_in.reshape(ET, 128).T
    wf32[:, OF_DTB : OF_DTB + D2T] = g["dt_proj_b"].reshape(D2T, 128).T
    wf32[:, OF_DCOL : OF_DCOL + D2T] = g["ssm_D"].reshape(D2T, 128).T
    wf32[:, OF_CFC1 : OF_CFC1 + HT] = c_fc1.reshape(HT, 128).T

    wbf = np.zeros((128, WBF_N), _BF)
    for i in range(D2T * KC):
        wbf[:, OB_DGX + i * 128 : OB_DGX + (i + 1) * 128] = diag_x[i].astype(_BF)
        wbf[:, OB_DGZ + i * 128 : OB_DGZ + (i + 1) * 128] = diag_z[i].astype(_BF)
    wbf[:, OB_XPJ : OB_XPJ + D2T * (R + 2 * NS)] = packrows(
        bf(g["x_proj_w"].T), D2T
    )
    for q in range(16):
        wbf[:, OB_SEL + q * 128 : OB_SEL + (q + 1) * 128] = sel[q].astype(_BF)
    wbf[:, OB_OPT : OB_OPT + ET * D] = packrows(bf(g["out_proj_w"].T), ET)
    wbf[:, OB_FC1 : OB_FC1 + DC * H] = packrows(bf(fc1.T), DC)
    wbf[:, OB_FC2 : OB_FC2 + HT * D] = packrows(bf(g["fc2_w"].T), HT)
    wbf[:, OB_IDB : OB_IDB + 128] = np.eye(128, dtype=np.float32).astype(_BF)
    for q in range(16):
        wbf[:, OB_REP + q * 128 : OB_REP + (q + 1) * 128] = rep[q].astype(_BF)
    dcol = g["ssm_D"].reshape(D2T, 128)
    for i in range(D2T):
        wbf[:, OB_DGD + i * 128 : OB_DGD + (i + 1) * 128] = np.diag(dcol[i]).astype(_BF)
    # zpad cols stay zero

    f = np.ascontiguousarray
    shared = {
        "wfr": f(wfr),
        "wf32": f(wf32),
        "wbf": f(wbf),
        "dt_projT": f(g["dt_proj_w"].T),
        "fc2b": f(g["fc2_b"].reshape(1, D)),
        "ones1d": np.ones((1, 128), np.float32),
    }
    return shared


_CACHED_NC = None


def kernel(**inputs):
    global _CACHED_NC
    from concourse.bass_utils import run_bass_kernel_spmd

    if _CACHED_NC is None:
        _CACHED_NC = build_kernel()
    nc = _CACHED_NC

    shared = prep_inputs(inputs)
    x = np.asarray(inputs["x"], dtype=np.float32)
    in_maps = [
        dict(shared, xin=np.ascontiguousarray(np.concatenate([x[i]] * KREPEAT, axis=0)))
        for i in range(NCORES)
    ]
    res = run_bass_kernel_spmd(nc, in_maps, core_ids=list(range(NCORES)))
    out = np.stack([res.results[i]["out"][:L] for i in range(NCORES)], axis=0)
    return out


if __name__ == "__main__":
    nc = build_kernel()
    print("build ok")
